# revision 18
# baseline (speedup 1.0000x reference)
"""Trainium2 Bass kernel for nn_DisentangledSelfAttentionWeighted.

Reference math (per sample, L=128, E=A=256, H=4, D=64):
    q = xq@Wq+bq, k = xk@Wk+bk, v = xv@Wv+bv, qp = xq@Ww+bw   (split into H heads of D)
    pair  = softmax_m( (q-mean_l q) . (k-mean_l k) )          [H,L,L]
    un    = softmax_m( k . mean_l qp )                        [H,L]
    out   = (pair + un) @ v + xq@Wr + br

Algebraic restructuring used here (exact, up to fp rounding):
  * softmax over m is shift-invariant in anything constant over m, so
    logits == q0_l.k0_m - (mu_q . k0_m), with q0/k0 the bias-FREE projections
    and mu_q = Wq^T mean_l(xq).  bq/bk drop entirely.
  * un logits == k0_m . mu_qp with mu_qp = Ww^T mean_l(xq) + bw (bk drops).
    The whole qp projection is never materialized.
  * sum_m softmax == 1 twice (pair and un), so bv enters the output twice:
    out += br + 2*bv, folded into one rank-1 broadcast matmul.
  * softmax normalization is deferred: exp goes unnormalized through the
    attention@V matmul (with a ones column producing the denominator Z per
    row), and 1/Z is applied per-partition during PSUM evacuation.

Layouts per sample (partition dim first):
  xT chunks [E/2=128, L]  (via DMA xbar transpose of bf16 input, or PE transpose)
  qT/kT     [A/2=128, L]  per A-chunk; heads = 64-row slices  -> attention
  pairT     [L_k, L_q] in PSUM; exp with per-partition bias -c[m]
  v         [L, H, D+1] with a ones column per head
  out       [L, A]
"""

import os
import sys
from contextlib import ExitStack

import numpy as np

sys.path.insert(0, "/opt/trn_rl_repo")

import ml_dtypes  # noqa: E402
import concourse.bass as bass  # noqa: E402
import concourse.tile as tile  # noqa: E402
from concourse import mybir  # noqa: E402

BF16 = mybir.dt.bfloat16
F32 = mybir.dt.float32
AF = mybir.ActivationFunctionType
ALU = mybir.AluOpType

B, L, E, A, H = 1024, 128, 256, 256, 4
D = A // H
NCORES = 8
NB = B // NCORES  # samples per core
G = 4  # samples per group (DMA/pipeline granularity)

# "mm":  transpose via regular matmul x.T @ I (bf16, x stationary)
# "dma": transpose inputs with the DMA xbar (bf16, offloads TensorE)
# "pe":  transpose inputs on TensorE transpose-mode (fp32) + evac
TRANSPOSE_MODE = os.environ.get("KERNEL_TRANSPOSE", "mm")


def _prep_w(w):
    # [E, A] f32 -> [128, 2, A] bf16  (partition = E within chunk)
    return np.ascontiguousarray(
        w.reshape(2, 128, A).transpose(1, 0, 2).astype(ml_dtypes.bfloat16)
    )


def prep_consts(Wq, bq, Wk, bk, Wv, bv, Ww, bw, Wr, br):
    """Host-side constant preparation (shared across cores)."""
    consts = {
        "WqB": _prep_w(Wq),
        "WkB": _prep_w(Wk),
        "WvB": _prep_w(Wv),
        "WwB": _prep_w(Ww),
        "WrB": _prep_w(Wr),
        # bw as per-partition columns for the two A-chunks
        "bw2": np.ascontiguousarray(bw.reshape(2, 128).T.astype(np.float32)),
        # (br + 2*bv)/4 replicated over 4 partitions (summed by ones4 matmul)
        "brbv4": np.ascontiguousarray(
            np.tile((br + 2.0 * bv)[None, :] / 4.0, (4, 1)).astype(ml_dtypes.bfloat16)
        ),
        "ones4": np.ones((4, L), dtype=ml_dtypes.bfloat16),
        # mask4[h', 64h+j] = (h'==h): picks diagonal blocks of the uv cross product
        "mask4": np.kron(np.eye(4), np.ones((1, D))).astype(ml_dtypes.bfloat16),
    }
    return consts


def split_excess_waits(nc):
    """Walrus/ISA allows ONE sync wait per engine instruction; Tile sometimes
    emits more.  Move excess waits onto same-engine NOPs inserted just before
    the offending instruction (engine queues execute in program order)."""
    import bass_rust

    engmap = {e.engine: e for e in nc.engines.values()}
    for f in nc.m.functions:
        for b in f.blocks:
            il = b.instructions
            idx = 0
            while idx < len(il):
                inst = il[idx]
                si = inst.sync_info
                if (
                    si is not None
                    and len(si.on_wait) > 1
                    and type(inst).__name__ not in ("InstEventSemaphore",)
                ):
                    waits = list(si.on_wait)
                    keep = waits[-1]
                    inst.sync_info = bass_rust.SyncInfo(
                        on_wait=[keep], on_update=list(si.on_update)
                    )
                    for w in waits[:-1]:
                        nop_bi = engmap[inst.engine].nop()
                        nop = nop_bi.ins
                        # remove the freshly-appended nop from wherever the
                        # builder put it
                        for bb in nc.m.functions[0].blocks:
                            lst = bb.instructions
                            if lst and lst[-1] is nop:
                                lst.pop()
                                break
                        nop.sync_info = bass_rust.SyncInfo(
                            on_wait=[w], on_update=[]
                        )
                        il.insert(idx, nop)
                        idx += 1
                idx += 1


def build_nc(nb=NB, g=G, transpose_mode=TRANSPOSE_MODE, repeats=1):
    """Build the per-core Bass program for nb samples.

    repeats>1 re-runs the whole computation (same I/O) for timing: the
    difference between repeats=2 and repeats=1 wall-clock is one pass of
    pure device execution with transfer/dispatch overheads cancelled.
    """
    assert nb % g == 0
    ngrp = nb // g
    nc = bass.Bass("TRN2", debug=False)

    # ---- DRAM I/O ----
    dq = nc.dram_tensor("query", [nb, L, E], F32, kind="ExternalInput").ap()
    dk = nc.dram_tensor("key", [nb, L, E], F32, kind="ExternalInput").ap()
    dv = nc.dram_tensor("value", [nb, L, E], F32, kind="ExternalInput").ap()
    dW = {
        n: nc.dram_tensor(n, [128, 2, A], BF16, kind="ExternalInput").ap()
        for n in ("WqB", "WkB", "WvB", "WwB", "WrB")
    }
    dbw2 = nc.dram_tensor("bw2", [128, 2], F32, kind="ExternalInput").ap()
    dbrbv4 = nc.dram_tensor("brbv4", [4, A], BF16, kind="ExternalInput").ap()
    dones4 = nc.dram_tensor("ones4", [4, L], BF16, kind="ExternalInput").ap()
    dmask4 = nc.dram_tensor("mask4", [4, A], BF16, kind="ExternalInput").ap()
    dout = nc.dram_tensor("out", [nb, L, A], F32, kind="ExternalOutput").ap()

    with tile.TileContext(nc) as tc, ExitStack() as ctx:
        const = ctx.enter_context(tc.tile_pool(name="const", bufs=1))
        xin = ctx.enter_context(tc.tile_pool(name="xin", bufs=2))
        xbfp = ctx.enter_context(tc.tile_pool(name="xbf", bufs=2))
        xtp = ctx.enter_context(tc.tile_pool(name="xt", bufs=2))
        projp = ctx.enter_context(tc.tile_pool(name="proj", bufs=2))
        smallp = ctx.enter_context(tc.tile_pool(name="small", bufs=2))
        outp = ctx.enter_context(tc.tile_pool(name="outs", bufs=2))
        ps_proj = ctx.enter_context(tc.tile_pool(name="ps_proj", bufs=3, space="PSUM"))
        ps_pair = ctx.enter_context(tc.tile_pool(name="ps_pair", bufs=2, space="PSUM"))
        ps_r = ctx.enter_context(tc.tile_pool(name="ps_r", bufs=1, space="PSUM"))
        ps_av = ctx.enter_context(tc.tile_pool(name="ps_av", bufs=2, space="PSUM"))

        # ---- constants to SBUF ----
        W = {}
        for n in ("WqB", "WkB", "WvB", "WwB", "WrB"):
            W[n] = const.tile([128, 2, A], BF16, name=n + "_sb")
            nc.sync.dma_start(out=W[n], in_=dW[n])
        bw_sb = const.tile([128, 2], F32)
        nc.sync.dma_start(out=bw_sb, in_=dbw2)
        brbv_sb = const.tile([4, A], BF16)
        nc.sync.dma_start(out=brbv_sb, in_=dbrbv4)
        ones4_sb = const.tile([4, L], BF16)
        nc.sync.dma_start(out=ones4_sb, in_=dones4)
        mask4_sb = const.tile([4, A], BF16)
        nc.sync.dma_start(out=mask4_sb, in_=dmask4)
        if transpose_mode == "pe":
            from concourse.masks import make_identity

            ident = const.tile([128, 128], F32)
            make_identity(nc, ident)
        elif transpose_mode == "mm":
            from concourse.masks import make_identity

            ident = const.tile([128, 128], BF16)
            make_identity(nc, ident)

        for grp in range(ngrp * repeats):
            grp = grp % ngrp
            s0 = grp * g
            # ---- load inputs (DRAM [g, L, E] -> SBUF [L, g, E]) ----
            q_nat = xin.tile([L, g, E], F32, tag="q_nat")
            k_nat = xin.tile([L, g, E], F32, tag="k_nat")
            v_nat = xin.tile([L, g, E], F32, tag="v_nat")
            nc.sync.dma_start(out=q_nat, in_=dq[s0 : s0 + g].rearrange("g l e -> l g e"))
            nc.sync.dma_start(out=k_nat, in_=dk[s0 : s0 + g].rearrange("g l e -> l g e"))
            nc.sync.dma_start(out=v_nat, in_=dv[s0 : s0 + g].rearrange("g l e -> l g e"))

            # transposed bf16 inputs: xT[input][chunk] = [E-chunk(128), g, L]
            xT = {
                n: [xtp.tile([128, g, L], BF16, tag=f"{n}T{c}", name=f"{n}T{c}")
                    for c in range(2)]
                for n in ("q", "k", "v")
            }
            mq_f = smallp.tile([128, 2, g], F32, tag="mq_f")
            if transpose_mode == "dma":
                # Cast to bf16 on gpsimd, pre-touch dst tiles on gpsimd, then
                # DMA-xbar transpose.  All transpose deps funnel through the
                # gpsimd semaphore so each DmaTranspose carries ONE sync wait
                # (walrus limit on the XPOSE descriptor).
                for n, nat in (("q", q_nat), ("k", k_nat), ("v", v_nat)):
                    xbf = xbfp.tile([L, g, E], BF16, tag=f"{n}bf", name=f"{n}bf")
                    nc.gpsimd.tensor_copy(xbf, nat)
                    for c in range(2):
                        nc.gpsimd.memset(xT[n][c], 0.0)
                    for s in range(g):
                        for c in range(2):
                            nc.scalar.dma_start_transpose(
                                out=xT[n][c][:, s, :],
                                in_=xbf[:, s, c * 128 : (c + 1) * 128],
                            )
                # mean of query over L
                for c in range(2):
                    nc.vector.tensor_reduce(
                        mq_f[:, c, :], xT["q"][c], axis=mybir.AxisListType.X,
                        op=ALU.add,
                    )
            elif transpose_mode == "mm":
                # transpose = x.T @ I as a REGULAR matmul (x stationary, bf16).
                # Regular matmuls split waits across LDW/MM structs, dodging
                # the single-sync-wait limit of transpose-mode/DMA-xpose.
                for n, nat in (("q", q_nat), ("k", k_nat), ("v", v_nat)):
                    xbf = xbfp.tile([L, g, E], BF16, tag=f"{n}bf", name=f"{n}bf")
                    nc.vector.tensor_copy(xbf, nat)
                    for s in range(g):
                        for c in range(2):
                            tp_ps = ps_proj.tile(
                                [128, 128], F32, tag="proj", name="tp_ps"
                            )
                            nc.tensor.matmul(
                                tp_ps,
                                lhsT=xbf[:, s, c * 128 : (c + 1) * 128],
                                rhs=ident,
                                start=True,
                                stop=True,
                            )
                            if n == "q":
                                nc.scalar.activation(
                                    xT[n][c][:, s, :], tp_ps, AF.Copy,
                                    accum_out=mq_f[:, c, s : s + 1],
                                )
                            else:
                                nc.vector.tensor_copy(xT[n][c][:, s, :], tp_ps)
            else:
                for n, nat in (("q", q_nat), ("k", k_nat), ("v", v_nat)):
                    for s in range(g):
                        for c in range(2):
                            tp_ps = ps_proj.tile(
                                [128, 128], F32, tag="proj", name="tp_ps"
                            )
                            nc.tensor.transpose(
                                tp_ps, nat[:, s, c * 128 : (c + 1) * 128], ident
                            )
                            if n == "q":
                                # free running sum over L -> per-sample mean
                                nc.scalar.activation(
                                    xT[n][c][:, s, :], tp_ps, AF.Copy,
                                    accum_out=mq_f[:, c, s : s + 1],
                                )
                            else:
                                nc.vector.tensor_copy(xT[n][c][:, s, :], tp_ps)

            mq_bf = smallp.tile([128, 2, g], BF16, tag="mq_bf")
            nc.vector.tensor_scalar(mq_bf, mq_f, 1.0 / L, None, op0=ALU.mult)

            # mu_q = Wq^T mq ; mu_qp = Ww^T mq + bw      [A(2 chunks of 128), g]
            mu_ps = ps_proj.tile([128, 2, 2, g], F32, tag="proj", name="mu_ps")
            for ac in range(2):
                for kind, wn in ((0, "WqB"), (1, "WwB")):
                    for ec in range(2):
                        nc.tensor.matmul(
                            mu_ps[:, ac, kind, :],
                            lhsT=W[wn][:, ec, ac * 128 : (ac + 1) * 128],
                            rhs=mq_bf[:, ec, :],
                            start=(ec == 0),
                            stop=(ec == 1),
                        )

            # ---- projections + attention per sample ----
            # QP: [A-chunk, g, 132]: cols 0:128 = qT, 128 = mu_q, 129 = mu_qp
            QP = [projp.tile([128, g, 132], BF16, tag=f"QP{ac}", name=f"QP{ac}")
                  for ac in range(2)]
            KP = [projp.tile([128, g, L], BF16, tag=f"KP{ac}", name=f"KP{ac}")
                  for ac in range(2)]
            # V: [L, g, H, D+1] with ones in col D
            V = projp.tile([L, g, H, D + 1], BF16, tag="V")
            nc.vector.memset(V[:, :, :, D : D + 1], 1.0)
            out_sb = outp.tile([L, g, A], F32, tag="out_sb")

            for ac in range(2):
                nc.vector.tensor_copy(QP[ac][:, :, 128:129], mu_ps[:, ac, 0:1, :].rearrange("p k g -> p g k"))
                nc.vector.tensor_scalar(
                    QP[ac][:, :, 129:130],
                    mu_ps[:, ac, 1:2, :].rearrange("p k g -> p g k"),
                    bw_sb[:, ac : ac + 1],
                    None,
                    op0=ALU.add,
                )

            for s in range(g):
                # q/k projections -> [A-chunk, L] (transposed layout)
                q_ps = ps_proj.tile([128, 2, 128], F32, tag="proj", name="q_ps")
                for ac in range(2):
                    for ec in range(2):
                        nc.tensor.matmul(
                            q_ps[:, ac, :],
                            lhsT=W["WqB"][:, ec, ac * 128 : (ac + 1) * 128],
                            rhs=xT["q"][ec][:, s, :],
                            start=(ec == 0),
                            stop=(ec == 1),
                        )
                for ac in range(2):
                    nc.scalar.copy(QP[ac][:, s, 0:128], q_ps[:, ac, :])

                k_ps = ps_proj.tile([128, 2, 128], F32, tag="proj", name="k_ps")
                for ac in range(2):
                    for ec in range(2):
                        nc.tensor.matmul(
                            k_ps[:, ac, :],
                            lhsT=W["WkB"][:, ec, ac * 128 : (ac + 1) * 128],
                            rhs=xT["k"][ec][:, s, :],
                            start=(ec == 0),
                            stop=(ec == 1),
                        )
                for ac in range(2):
                    nc.scalar.copy(KP[ac][:, s, :], k_ps[:, ac, :])

                # v projection -> natural [L, A]
                v_ps = ps_proj.tile([L, A], F32, tag="proj", name="v_ps")
                for ec in range(2):
                    nc.tensor.matmul(
                        v_ps,
                        lhsT=xT["v"][ec][:, s, :],
                        rhs=W["WvB"][:, ec, :],
                        start=(ec == 0),
                        stop=(ec == 1),
                    )
                nc.vector.tensor_copy(
                    V[:, s, :, 0:D], v_ps.rearrange("l (h d) -> l h d", h=H)
                )

                # pair logits (transposed) + c + u columns, per head
                # pairT[m, l] = k0T_h . q0T_h ; col 128 = c[m] ; col 129 = u[m]
                pair_ps = [
                    ps_pair.tile([128, 130], F32, tag="pair", name=f"pair{h}")
                    for h in range(H)
                ]
                cu_sb = smallp.tile([128, H, 2], F32, tag="cu")
                for h in range(H):
                    ac, hh = h // 2, h % 2
                    off = hh * 64
                    nc.tensor.matmul(
                        pair_ps[h],
                        lhsT=KP[ac][off : off + 64, s, 0:128],
                        rhs=QP[ac][off : off + 64, s, 0:130],
                        start=True,
                        stop=True,
                    )
                    # negate c,u into SBUF (exp bias needs SBUF per-part AP)
                    nc.vector.tensor_scalar(
                        cu_sb[:, h, :], pair_ps[h][:, 128:130], -1.0, None,
                        op0=ALU.mult,
                    )

                # exp(pair - c) -> bf16 ; exp(u) -> bf16
                expT = smallp.tile([128, H, 128], BF16, tag="expT")
                for h in range(H):
                    nc.scalar.activation(
                        expT[:, h, :],
                        pair_ps[h][:, 0:128],
                        AF.Exp,
                        bias=cu_sb[:, h, 0:1],
                    )
                exu = smallp.tile([128, H], BF16, tag="exu")
                nc.scalar.activation(
                    exu, cu_sb[:, :, 1], AF.Exp, scale=-1.0
                )

                # uv cross product [4, 4*D] + Zu at col 256
                uvz_ps = ps_av.tile([4, A + 1], F32, tag="av", name="uvz_ps")
                nc.tensor.matmul(
                    uvz_ps[:, 0:A], lhsT=exu, rhs=V[:, s, :, 0:D],
                    start=True, stop=True,
                )
                nc.tensor.matmul(
                    uvz_ps[:, A : A + 1], lhsT=exu, rhs=V[:, s, 0, D : D + 1],
                    start=True, stop=True,
                )
                ruz = smallp.tile([4, 1], F32, tag="ruz")
                nc.vector.reciprocal(ruz, uvz_ps[:, A : A + 1])
                uv_sb = smallp.tile([4, A], BF16, tag="uv_sb")
                nc.vector.scalar_tensor_tensor(
                    uv_sb, uvz_ps[:, 0:A], ruz, mask4_sb,
                    op0=ALU.mult, op1=ALU.mult,
                )

                # residual r = xq @ Wr, then += broadcast(uv + br + 2bv)
                r_ps = ps_r.tile([L, A], F32, tag="r")
                for ec in range(2):
                    nc.tensor.matmul(
                        r_ps,
                        lhsT=xT["q"][ec][:, s, :],
                        rhs=W["WrB"][:, ec, :],
                        start=(ec == 0),
                        stop=False,
                    )
                nc.tensor.matmul(
                    r_ps, lhsT=ones4_sb, rhs=uv_sb, start=False, stop=False
                )
                nc.tensor.matmul(
                    r_ps, lhsT=ones4_sb, rhs=brbv_sb, start=False, stop=True
                )
                r_sb = smallp.tile([L, A], F32, tag="r_sb")
                nc.scalar.copy(r_sb, r_ps)

                # attention @ V (+ ones col -> Zp), then normalize + add r
                av_ps = ps_av.tile([L, H, D + 1], F32, tag="av", name="av_ps")
                for h in range(H):
                    nc.tensor.matmul(
                        av_ps[:, h, :], lhsT=expT[:, h, :], rhs=V[:, s, h, :],
                        start=True, stop=True,
                    )
                rzp = smallp.tile([L, H], F32, tag="rzp")
                nc.vector.reciprocal(rzp, av_ps[:, :, D])
                for h in range(H):
                    nc.vector.scalar_tensor_tensor(
                        out_sb[:, s, h * D : (h + 1) * D],
                        av_ps[:, h, 0:D],
                        rzp[:, h : h + 1],
                        r_sb[:, h * D : (h + 1) * D],
                        op0=ALU.mult,
                        op1=ALU.add,
                    )
            nc.sync.dma_start(
                out=dout[s0 : s0 + g].rearrange("g l a -> l g a"), in_=out_sb
            )
    split_excess_waits(nc)
    return nc


_NC_CACHE = {}


def _get_nc(nb=NB):
    if nb not in _NC_CACHE:
        _NC_CACHE[nb] = build_nc(nb)
    return _NC_CACHE[nb]


def kernel(query, key, value, Wq, bq, Wk, bk, Wv, bv, Ww, bw, Wr, br):
    query = np.asarray(query, dtype=np.float32)
    key = np.asarray(key, dtype=np.float32)
    value = np.asarray(value, dtype=np.float32)
    consts = prep_consts(
        np.asarray(Wq, np.float32), np.asarray(bq, np.float32),
        np.asarray(Wk, np.float32), np.asarray(bk, np.float32),
        np.asarray(Wv, np.float32), np.asarray(bv, np.float32),
        np.asarray(Ww, np.float32), np.asarray(bw, np.float32),
        np.asarray(Wr, np.float32), np.asarray(br, np.float32),
    )
    nc = _get_nc(NB)
    from concourse.bass_utils import run_bass_kernel_spmd

    in_maps = []
    for core in range(NCORES):
        sl = slice(core * NB, (core + 1) * NB)
        m = {"query": query[sl], "key": key[sl], "value": value[sl]}
        m.update(consts)
        in_maps.append(m)
    res = run_bass_kernel_spmd(nc, in_maps, core_ids=list(range(NCORES)))
    out = np.concatenate([r["out"] for r in res.results], axis=0)
    return out.astype(np.float32)


if __name__ == "__main__":
    rng = np.random.default_rng(0)
    s = 0.02
    inputs = {
        "query": rng.standard_normal((B, L, E), dtype=np.float32),
        "key": rng.standard_normal((B, L, E), dtype=np.float32),
        "value": rng.standard_normal((B, L, E), dtype=np.float32),
    }
    for n in ("q", "k", "v", "w", "r"):
        inputs["W" + n] = rng.standard_normal((E, A), dtype=np.float32) * s
        inputs["b" + n] = rng.standard_normal((A,), dtype=np.float32) * s
    out = kernel(**inputs)
    print("out", out.shape, out.dtype, float(np.abs(out).max()))
